# revision 63
# baseline (speedup 1.0000x reference)
"""LorentzLinear (geoopt expmap0 + Minkowski GEMM) on 8 Trainium2 NeuronCores.

Math: out[b,o] = <expmap0(x)[b] * sign, expmap0(w)[o]> + bias, sign = [-1,1,..,1].

With per-row stats (nomin = sqrt(clamp(<u,u>_L, eps)), alpha = sinh(nomin)/nomin,
e0 = cosh(nomin) + alpha*u0) the output factors exactly as

    out[b,o] = ax[b]*aw[o]*G0[b,o] - ux[b]*vw[o] - ex0[b]*ew0[o] + bias[o]

where G0 = x @ w.T is a plain GEMM, ux = ax*x[:,0], vw = aw*w[:,0].

Sharding: data-parallel along batch. Each core runs the full-K GEMM for its
1024 batch rows (M=1024, K=2048, N=2048 on the PE, fp16 operands / fp32 PSUM
accumulate — the GEMM term contributes < 1e-4 of the output scale, see below)
plus a per-[128,512]-tile epilogue: ACT drains PSUM with the ax row-scale,
then three DVE ops apply aw and the rank-2 correction. The correction
products (which carry the fp32-overflow structure of the reference output,
|ex0*ew0| ~ 1e38..1e40) are computed in IEEE fp32 on ACT/DVE, so the -inf
pattern of the reference is reproduced to ulp level; the inf/NaN mismatch
floor (~0.24% of entries) is the reference's own fp32 accumulation-order
sensitivity, irreproducible by any reordered implementation.

Host side does only data movement (transpose/shard) and the per-row stats
(row norms + sinh/cosh on 10k rows, 0.25% of total FLOPs) — stats are done on
host so the transcendentals match the CPU reference's libm to ~1 ulp; the
reference output's inf/finite boundary is exponentially sensitive to them.
"""

import os
import numpy as np

os.environ.pop("BASS_TRACE", None)  # tracing needs an NTFF hook this image lacks

B, D, O = 8192, 2048, 2048
NCORES = 8
BS = B // NCORES           # batch rows per core
MT, NT, KT = 128, 512, 128  # matmul tile dims
NM, NN, NK = BS // MT, O // NT, D // KT

GEMM_DTYPE = os.environ.get("KERNEL_GEMM_DTYPE", "float16")

_cache = {}


def _build_nc(with_bias: bool, gemm_dtype: str):
    import concourse.tile as tile
    from concourse import bacc, mybir

    F32 = mybir.dt.float32
    GD = getattr(mybir.dt, gemm_dtype)
    MUL, ADD = mybir.AluOpType.mult, mybir.AluOpType.add

    nc = bacc.Bacc("TRN2", target_bir_lowering=False, debug=False,
                   num_devices=NCORES)
    xT_d = nc.dram_tensor("xT", [D, BS], GD, kind="ExternalInput").ap()
    wT_d = nc.dram_tensor("wT", [D, O], GD, kind="ExternalInput").ap()
    ax_d = nc.dram_tensor("ax_pp", [MT, NM], F32, kind="ExternalInput").ap()
    ux_d = nc.dram_tensor("ux_pp", [MT, NM], F32, kind="ExternalInput").ap()
    ex0_d = nc.dram_tensor("ex0_pp", [MT, NM], F32, kind="ExternalInput").ap()
    aw_d = nc.dram_tensor("aw_row", [1, O], F32, kind="ExternalInput").ap()
    nvw_d = nc.dram_tensor("nvw_row", [1, O], F32, kind="ExternalInput").ap()
    ne0_d = nc.dram_tensor("ne0_row", [1, O], F32, kind="ExternalInput").ap()
    if with_bias:
        bias_d = nc.dram_tensor("bias_row", [1, O], F32, kind="ExternalInput").ap()
    out_d = nc.dram_tensor("out", [BS, O], F32, kind="ExternalOutput").ap()

    with tile.TileContext(nc) as tc:
        MH = 1024  # x columns per xs tile — 2KB DMA rows
        NH = BS // MH
        with tc.tile_pool(name="xs", bufs=1) as xs_pool, \
             tc.tile_pool(name="ws", bufs=1) as ws_pool, \
             tc.tile_pool(name="vec", bufs=1) as vec_pool, \
             tc.tile_pool(name="ep", bufs=3) as ep_pool, \
             tc.tile_pool(name="ep0", bufs=10) as ep0_pool, \
             tc.tile_pool(name="ps", bufs=6, space="PSUM") as ps_pool, \
             tc.tile_pool(name="wu", bufs=1, space="PSUM") as wu_pool:

            # DMA issue order is the startup latency: the first PSUM group
            # needs (ws[n=0][k], xs[k][half0]) pairs in k order — issue exactly
            # those first so the PE can chase the DMA stream.
            ws_t = {}
            xs_t = {}

            def load_ws(n):
                # one tile per n-chunk: [128, k*512] with k-slices via AP
                # offsets — far fewer tiles means far less semaphore/release
                # traffic on the PE instruction stream
                t = ws_pool.tile([KT, NK * NT], GD, tag=f"ws{n}")
                nc.sync.dma_start(
                    t[:],
                    wT_d[:, n * NT:(n + 1) * NT].rearrange(
                        "(k p) n -> p k n", p=KT))
                ws_t[n] = t

            def load_xs(k, h):
                t = xs_pool.tile([KT, MH], GD, tag=f"xs{k}_{h}")
                nc.sync.dma_start(
                    t[:], xT_d[k * KT:(k + 1) * KT, h * MH:(h + 1) * MH])
                xs_t[(k, h)] = t

            # smallest-possible first-matmul dependency: k=0 of x is split
            # into per-m [128,128] tiles and issued before everything else,
            # so the real stream can start as soon as ~160KB has landed
            xs0m_t = []
            t = ws_pool.tile([KT, NT], GD, tag="ws0_0")
            nc.sync.dma_start(t[:], wT_d[0:KT, 0:NT])
            ws0_first = t
            for m in range(NM):
                t = xs_pool.tile([KT, MT], GD, tag=f"xs0m{m}")
                nc.sync.dma_start(t[:], xT_d[0:KT, m * MT:(m + 1) * MT])
                xs0m_t.append(t)

            # ax next: the ACT psum-drain only needs this 12KB vector, so the
            # PSUM pipeline can drain from the first group onwards
            ax_t = vec_pool.tile([MT, NM], F32, tag="ax")
            nc.sync.dma_start(ax_t[:], ax_d)

            # HAM warm-up: a short dummy burst bridges the PE to the real
            # stream start; the first few real matmuls still run at half
            # clock until the HAM window fills (~3.4us of sustained busy)
            wu = vec_pool.tile([KT, NT], GD, tag="warmup")
            nc.vector.memset(wu[:], 0.0)
            wu_ps = wu_pool.tile([MT, NT], mybir.dt.float32, tag="wu_ps")
            for _ in range(4):
                nc.tensor.matmul(wu_ps[:], wu[:, :MT], wu[:],
                                 start=True, stop=True)
            wu_out = vec_pool.tile([MT, 1], F32, tag="wu_out")
            nc.vector.tensor_copy(wu_out[:], wu_ps[:, :1])

            # ramp order: the PE consumes (ws[0][k], xs[k][h0]) pairs first,
            # then xs halves 1.., then ws n=1 — finer interleave keeps the
            # chase deficit minimal
            # n=0 stays fine-grained per k so the PE can chase the DMA stream
            # during the ramp; later n-chunks are single tiles (less semaphore
            # traffic on the PE pipe)
            ws0_t = {0: ws0_first}
            for k in range(1, NK):
                t = ws_pool.tile([KT, NT], GD, tag=f"ws0_{k}")
                nc.sync.dma_start(
                    t[:], wT_d[k * KT:(k + 1) * KT, 0:NT])
                ws0_t[k] = t
                for h in range(NH):
                    load_xs(k, h)
            load_ws(1)

            # remaining epilogue vectors: rows in, broadcast across partitions
            # on GpSimd (otherwise idle)
            ux_t = vec_pool.tile([MT, NM], F32, tag="ux")
            ex0_t = vec_pool.tile([MT, NM], F32, tag="ex0")
            nc.sync.dma_start(ux_t[:], ux_d)
            nc.sync.dma_start(ex0_t[:], ex0_d)

            def bcast(tag, row_d):
                row_t = vec_pool.tile([1, O], F32, tag=f"{tag}_row")
                nc.sync.dma_start(row_t[:], row_d)
                full_t = vec_pool.tile([MT, O], F32, tag=tag)
                nc.gpsimd.partition_broadcast(full_t[:], row_t[:])
                return full_t

            awB_t = bcast("awB", aw_d)
            nvwB_t = bcast("nvwB", nvw_d)
            ne0B_t = bcast("ne0B", ne0_d)

            # prefetch remaining ws chunks (bufs: all four stay resident)
            for n in range(2, NN):
                load_ws(n)
            if with_bias:
                biasB_t = bcast("biasB", bias_d)

            for n in range(NN):
                for m in range(NM):
                    h, hm = divmod(m, MH // MT)
                    psum = ps_pool.tile([MT, NT], mybir.dt.float32)
                    for k in range(NK):
                        rhs = (ws0_t[k][:] if n == 0
                               else ws_t[n][:, k * NT:(k + 1) * NT])
                        lhsT = (xs0m_t[m][:] if k == 0 else
                                xs_t[(k, h)][:, hm * MT:(hm + 1) * MT])
                        nc.tensor.matmul(
                            psum[:],
                            lhsT,
                            rhs,
                            start=(k == 0), stop=(k == NK - 1),
                        )
                    ms = slice(m, m + 1)
                    # the very last tile's epilogue is a serial chain hanging
                    # off the final matmul — split it in half so the ACT/DVE
                    # stages pipeline and the tail shortens
                    last = (n == NN - 1 and m == NM - 1)
                    for c0, c1 in ([(0, NT // 2), (NT // 2, NT)] if last
                                   else [(0, NT)]):
                        w_ = c1 - c0
                        ns = slice(n * NT + c0, n * NT + c1)
                        ps_s = psum[:, c0:c1]
                        # ACT drains PSUM (needs only ax) so the PE never
                        # waits on PSUM banks even before the epilogue
                        # vectors arrive
                        t0 = ep0_pool.tile([MT, w_], F32, tag=f"t0_{w_}")
                        nc.scalar.mul(t0[:], ps_s, ax_t[:, ms])
                        t1 = ep_pool.tile([MT, w_], F32, tag=f"t1_{w_}")
                        nc.vector.tensor_mul(t1[:], t0[:], awB_t[:, ns])
                        t2 = ep_pool.tile([MT, w_], F32, tag=f"t2_{w_}")
                        nc.vector.scalar_tensor_tensor(
                            t2[:], nvwB_t[:, ns], ux_t[:, ms], t1[:], MUL, ADD)
                        t3 = ep_pool.tile([MT, w_], F32, tag=f"t3_{w_}")
                        nc.vector.scalar_tensor_tensor(
                            t3[:], ne0B_t[:, ns], ex0_t[:, ms], t2[:], MUL, ADD)
                        if with_bias:
                            t4 = ep_pool.tile([MT, w_], F32, tag=f"t4_{w_}")
                            nc.vector.tensor_add(t4[:], t3[:], biasB_t[:, ns])
                            t3 = t4
                        nc.sync.dma_start(
                            out_d[m * MT:(m + 1) * MT, ns], t3[:])
    nc.compile()
    return nc


def _row_stats(u: np.ndarray):
    """Per-row expmap0 stats in fp32, matching the reference's op order."""
    f32 = np.float32
    sq = (u[:, 1:].astype(f32)) ** 2
    msq = (-(u[:, 0].astype(f32) ** 2) + sq.sum(axis=1, dtype=f32)).astype(f32)
    nomin = np.sqrt(np.maximum(msq, f32(1e-8))).astype(f32)
    sh = np.sinh(nomin).astype(f32)
    ch = np.cosh(nomin).astype(f32)
    alpha = (sh / nomin).astype(f32)
    r0 = ((sh * u[:, 0].astype(f32)).astype(f32) / nomin).astype(f32)
    e0 = (ch + r0).astype(f32)
    u0scaled = (alpha * u[:, 0].astype(f32)).astype(f32)
    return alpha, u0scaled, e0


def prepare_inputs(x, weight, bias):
    """Host prep: stats, transpose, shard. Returns (in_maps, with_bias)."""
    x = np.ascontiguousarray(np.asarray(x, dtype=np.float32))
    w = np.ascontiguousarray(np.asarray(weight, dtype=np.float32))
    bias = np.asarray(bias, dtype=np.float32)
    assert x.shape == (B, D) and w.shape == (O, D) and bias.shape == (O,)

    ax, ux, ex0 = _row_stats(x)
    aw, vw, ew0 = _row_stats(w)

    gemm_np = np.float16 if GEMM_DTYPE == "float16" else np.float32
    xT = np.ascontiguousarray(x.T.astype(gemm_np))   # [D, B]
    wT = np.ascontiguousarray(w.T.astype(gemm_np))   # [D, O]
    aw_row = np.ascontiguousarray(aw.reshape(1, O))
    nvw_row = np.ascontiguousarray(-vw.reshape(1, O))
    ne0_row = np.ascontiguousarray(-ew0.reshape(1, O))
    with_bias = bool(np.any(bias != 0))
    bias_row = np.ascontiguousarray(bias.reshape(1, O)) if with_bias else None

    in_maps = []
    for c in range(NCORES):
        bs = slice(c * BS, (c + 1) * BS)
        m = {
            "xT": np.ascontiguousarray(xT[:, bs]),
            "wT": wT,
            "ax_pp": np.ascontiguousarray(ax[bs].reshape(NM, MT).T),
            "ux_pp": np.ascontiguousarray(ux[bs].reshape(NM, MT).T),
            "ex0_pp": np.ascontiguousarray(ex0[bs].reshape(NM, MT).T),
            "aw_row": aw_row,
            "nvw_row": nvw_row,
            "ne0_row": ne0_row,
        }
        if with_bias:
            m["bias_row"] = bias_row
        in_maps.append(m)
    return in_maps, with_bias


def get_nc(with_bias: bool):
    key = ("nc", with_bias, GEMM_DTYPE)
    if key not in _cache:
        _cache[key] = _build_nc(with_bias, GEMM_DTYPE)
    return _cache[key]


def kernel(x, weight, bias):
    from concourse.bass_utils import run_bass_kernel_spmd

    in_maps, with_bias = prepare_inputs(x, weight, bias)
    nc = get_nc(with_bias)
    res = run_bass_kernel_spmd(nc, in_maps, core_ids=list(range(NCORES)))
    out = np.concatenate([res.results[c]["out"] for c in range(NCORES)], axis=0)
    return out


# revision 64
# speedup vs baseline: 1.0284x; 1.0284x over previous
"""LorentzLinear (geoopt expmap0 + Minkowski GEMM) on 8 Trainium2 NeuronCores.

Math: out[b,o] = <expmap0(x)[b] * sign, expmap0(w)[o]> + bias, sign = [-1,1,..,1].

With per-row stats (nomin = sqrt(clamp(<u,u>_L, eps)), alpha = sinh(nomin)/nomin,
e0 = cosh(nomin) + alpha*u0) the output factors exactly as

    out[b,o] = ax[b]*aw[o]*G0[b,o] - ux[b]*vw[o] - ex0[b]*ew0[o] + bias[o]

where G0 = x @ w.T is a plain GEMM, ux = ax*x[:,0], vw = aw*w[:,0].

Sharding: data-parallel along batch. Each core runs the full-K GEMM for its
1024 batch rows (M=1024, K=2048, N=2048 on the PE, fp16 operands / fp32 PSUM
accumulate — the GEMM term contributes < 1e-4 of the output scale, see below)
plus a per-[128,512]-tile epilogue: ACT drains PSUM with the ax row-scale,
then three DVE ops apply aw and the rank-2 correction. The correction
products (which carry the fp32-overflow structure of the reference output,
|ex0*ew0| ~ 1e38..1e40) are computed in IEEE fp32 on ACT/DVE, so the -inf
pattern of the reference is reproduced to ulp level; the inf/NaN mismatch
floor (~0.24% of entries) is the reference's own fp32 accumulation-order
sensitivity, irreproducible by any reordered implementation.

Host side does only data movement (transpose/shard) and the per-row stats
(row norms + sinh/cosh on 10k rows, 0.25% of total FLOPs) — stats are done on
host so the transcendentals match the CPU reference's libm to ~1 ulp; the
reference output's inf/finite boundary is exponentially sensitive to them.
"""

import os
import numpy as np

os.environ.pop("BASS_TRACE", None)  # tracing needs an NTFF hook this image lacks

B, D, O = 8192, 2048, 2048
NCORES = 8
BS = B // NCORES           # batch rows per core
MT, NT, KT = 128, 512, 128  # matmul tile dims
NM, NN, NK = BS // MT, O // NT, D // KT

GEMM_DTYPE = os.environ.get("KERNEL_GEMM_DTYPE", "float16")

_cache = {}


def _build_nc(with_bias: bool, gemm_dtype: str):
    import concourse.tile as tile
    from concourse import bacc, mybir

    F32 = mybir.dt.float32
    GD = getattr(mybir.dt, gemm_dtype)
    MUL, ADD = mybir.AluOpType.mult, mybir.AluOpType.add

    nc = bacc.Bacc("TRN2", target_bir_lowering=False, debug=False,
                   num_devices=NCORES)
    xT_d = nc.dram_tensor("xT", [D, BS], GD, kind="ExternalInput").ap()
    wT_d = nc.dram_tensor("wT", [D, O], GD, kind="ExternalInput").ap()
    ax_d = nc.dram_tensor("ax_pp", [MT, NM], F32, kind="ExternalInput").ap()
    ux_d = nc.dram_tensor("ux_pp", [MT, NM], F32, kind="ExternalInput").ap()
    ex0_d = nc.dram_tensor("ex0_pp", [MT, NM], F32, kind="ExternalInput").ap()
    aw_d = nc.dram_tensor("aw_row", [1, O], F32, kind="ExternalInput").ap()
    nvw_d = nc.dram_tensor("nvw_row", [1, O], F32, kind="ExternalInput").ap()
    ne0_d = nc.dram_tensor("ne0_row", [1, O], F32, kind="ExternalInput").ap()
    if with_bias:
        bias_d = nc.dram_tensor("bias_row", [1, O], F32, kind="ExternalInput").ap()
    out_d = nc.dram_tensor("out", [BS, O], F32, kind="ExternalOutput").ap()

    with tile.TileContext(nc) as tc:
        MH = 1024  # x columns per xs tile — 2KB DMA rows
        NH = BS // MH
        with tc.tile_pool(name="xs", bufs=1) as xs_pool, \
             tc.tile_pool(name="ws", bufs=1) as ws_pool, \
             tc.tile_pool(name="vec", bufs=1) as vec_pool, \
             tc.tile_pool(name="ep", bufs=3) as ep_pool, \
             tc.tile_pool(name="ep0", bufs=10) as ep0_pool, \
             tc.tile_pool(name="ps", bufs=6, space="PSUM") as ps_pool, \
             tc.tile_pool(name="wu", bufs=1, space="PSUM") as wu_pool:

            # DMA issue order is the startup latency: the first PSUM group
            # needs (ws[n=0][k], xs[k][half0]) pairs in k order — issue exactly
            # those first so the PE can chase the DMA stream.
            ws_t = {}
            xs_t = {}

            def load_ws(n):
                # one tile per n-chunk: [128, k*512] with k-slices via AP
                # offsets — far fewer tiles means far less semaphore/release
                # traffic on the PE instruction stream
                t = ws_pool.tile([KT, NK * NT], GD, tag=f"ws{n}")
                nc.sync.dma_start(
                    t[:],
                    wT_d[:, n * NT:(n + 1) * NT].rearrange(
                        "(k p) n -> p k n", p=KT))
                ws_t[n] = t

            def load_xs(k, h):
                t = xs_pool.tile([KT, MH], GD, tag=f"xs{k}_{h}")
                nc.sync.dma_start(
                    t[:], xT_d[k * KT:(k + 1) * KT, h * MH:(h + 1) * MH])
                xs_t[(k, h)] = t

            # ax first: the ACT psum-drain only needs this 12KB vector, so the
            # PSUM pipeline can drain from the first group onwards
            ax_t = vec_pool.tile([MT, NM], F32, tag="ax")
            nc.sync.dma_start(ax_t[:], ax_d)

            # HAM warm-up: dummy matmuls on a memset tile keep the PE busy
            # during the initial DMA wait so the real stream starts at 2.4GHz
            # instead of paying ~10 cold matmuls at half clock
            wu = vec_pool.tile([KT, NT], GD, tag="warmup")
            nc.vector.memset(wu[:], 0.0)
            wu_ps = wu_pool.tile([MT, NT], mybir.dt.float32, tag="wu_ps")
            for _ in range(12):
                nc.tensor.matmul(wu_ps[:], wu[:, :MT], wu[:],
                                 start=True, stop=True)
            wu_out = vec_pool.tile([MT, 1], F32, tag="wu_out")
            nc.vector.tensor_copy(wu_out[:], wu_ps[:, :1])

            # ramp order: the PE consumes (ws[0][k], xs[k][h0]) pairs first,
            # then xs halves 1.., then ws n=1 — finer interleave keeps the
            # chase deficit minimal
            # n=0 stays fine-grained per k so the PE can chase the DMA stream
            # during the ramp; later n-chunks are single tiles (less semaphore
            # traffic on the PE pipe)
            ws0_t = {}
            for k in range(NK):
                t = ws_pool.tile([KT, NT], GD, tag=f"ws0_{k}")
                nc.sync.dma_start(
                    t[:], wT_d[k * KT:(k + 1) * KT, 0:NT])
                ws0_t[k] = t
                for h in range(NH):
                    load_xs(k, h)
            load_ws(1)

            # remaining epilogue vectors: rows in, broadcast across partitions
            # on GpSimd (otherwise idle)
            ux_t = vec_pool.tile([MT, NM], F32, tag="ux")
            ex0_t = vec_pool.tile([MT, NM], F32, tag="ex0")
            nc.sync.dma_start(ux_t[:], ux_d)
            nc.sync.dma_start(ex0_t[:], ex0_d)

            def bcast(tag, row_d):
                row_t = vec_pool.tile([1, O], F32, tag=f"{tag}_row")
                nc.sync.dma_start(row_t[:], row_d)
                full_t = vec_pool.tile([MT, O], F32, tag=tag)
                nc.gpsimd.partition_broadcast(full_t[:], row_t[:])
                return full_t

            awB_t = bcast("awB", aw_d)
            nvwB_t = bcast("nvwB", nvw_d)
            ne0B_t = bcast("ne0B", ne0_d)

            # prefetch remaining ws chunks (bufs: all four stay resident)
            for n in range(2, NN):
                load_ws(n)
            if with_bias:
                biasB_t = bcast("biasB", bias_d)

            for n in range(NN):
                for m in range(NM):
                    h, hm = divmod(m, MH // MT)
                    psum = ps_pool.tile([MT, NT], mybir.dt.float32)
                    for k in range(NK):
                        rhs = (ws0_t[k][:] if n == 0
                               else ws_t[n][:, k * NT:(k + 1) * NT])
                        nc.tensor.matmul(
                            psum[:],
                            xs_t[(k, h)][:, hm * MT:(hm + 1) * MT],
                            rhs,
                            start=(k == 0), stop=(k == NK - 1),
                        )
                    ms = slice(m, m + 1)
                    # the very last tile's epilogue is a serial chain hanging
                    # off the final matmul — split it in half so the ACT/DVE
                    # stages pipeline and the tail shortens
                    last = (n == NN - 1 and m == NM - 1)
                    for c0, c1 in ([(0, NT // 2), (NT // 2, NT)] if last
                                   else [(0, NT)]):
                        w_ = c1 - c0
                        ns = slice(n * NT + c0, n * NT + c1)
                        ps_s = psum[:, c0:c1]
                        # ACT drains PSUM (needs only ax) so the PE never
                        # waits on PSUM banks even before the epilogue
                        # vectors arrive
                        t0 = ep0_pool.tile([MT, w_], F32, tag=f"t0_{w_}")
                        nc.scalar.mul(t0[:], ps_s, ax_t[:, ms])
                        t1 = ep_pool.tile([MT, w_], F32, tag=f"t1_{w_}")
                        nc.vector.tensor_mul(t1[:], t0[:], awB_t[:, ns])
                        t2 = ep_pool.tile([MT, w_], F32, tag=f"t2_{w_}")
                        nc.vector.scalar_tensor_tensor(
                            t2[:], nvwB_t[:, ns], ux_t[:, ms], t1[:], MUL, ADD)
                        t3 = ep_pool.tile([MT, w_], F32, tag=f"t3_{w_}")
                        nc.vector.scalar_tensor_tensor(
                            t3[:], ne0B_t[:, ns], ex0_t[:, ms], t2[:], MUL, ADD)
                        if with_bias:
                            t4 = ep_pool.tile([MT, w_], F32, tag=f"t4_{w_}")
                            nc.vector.tensor_add(t4[:], t3[:], biasB_t[:, ns])
                            t3 = t4
                        nc.sync.dma_start(
                            out_d[m * MT:(m + 1) * MT, ns], t3[:])
    nc.compile()
    return nc


def _row_stats(u: np.ndarray):
    """Per-row expmap0 stats in fp32, matching the reference's op order."""
    f32 = np.float32
    sq = (u[:, 1:].astype(f32)) ** 2
    msq = (-(u[:, 0].astype(f32) ** 2) + sq.sum(axis=1, dtype=f32)).astype(f32)
    nomin = np.sqrt(np.maximum(msq, f32(1e-8))).astype(f32)
    sh = np.sinh(nomin).astype(f32)
    ch = np.cosh(nomin).astype(f32)
    alpha = (sh / nomin).astype(f32)
    r0 = ((sh * u[:, 0].astype(f32)).astype(f32) / nomin).astype(f32)
    e0 = (ch + r0).astype(f32)
    u0scaled = (alpha * u[:, 0].astype(f32)).astype(f32)
    return alpha, u0scaled, e0


def prepare_inputs(x, weight, bias):
    """Host prep: stats, transpose, shard. Returns (in_maps, with_bias)."""
    x = np.ascontiguousarray(np.asarray(x, dtype=np.float32))
    w = np.ascontiguousarray(np.asarray(weight, dtype=np.float32))
    bias = np.asarray(bias, dtype=np.float32)
    assert x.shape == (B, D) and w.shape == (O, D) and bias.shape == (O,)

    ax, ux, ex0 = _row_stats(x)
    aw, vw, ew0 = _row_stats(w)

    gemm_np = np.float16 if GEMM_DTYPE == "float16" else np.float32
    xT = np.ascontiguousarray(x.T.astype(gemm_np))   # [D, B]
    wT = np.ascontiguousarray(w.T.astype(gemm_np))   # [D, O]
    aw_row = np.ascontiguousarray(aw.reshape(1, O))
    nvw_row = np.ascontiguousarray(-vw.reshape(1, O))
    ne0_row = np.ascontiguousarray(-ew0.reshape(1, O))
    with_bias = bool(np.any(bias != 0))
    bias_row = np.ascontiguousarray(bias.reshape(1, O)) if with_bias else None

    in_maps = []
    for c in range(NCORES):
        bs = slice(c * BS, (c + 1) * BS)
        m = {
            "xT": np.ascontiguousarray(xT[:, bs]),
            "wT": wT,
            "ax_pp": np.ascontiguousarray(ax[bs].reshape(NM, MT).T),
            "ux_pp": np.ascontiguousarray(ux[bs].reshape(NM, MT).T),
            "ex0_pp": np.ascontiguousarray(ex0[bs].reshape(NM, MT).T),
            "aw_row": aw_row,
            "nvw_row": nvw_row,
            "ne0_row": ne0_row,
        }
        if with_bias:
            m["bias_row"] = bias_row
        in_maps.append(m)
    return in_maps, with_bias


def get_nc(with_bias: bool):
    key = ("nc", with_bias, GEMM_DTYPE)
    if key not in _cache:
        _cache[key] = _build_nc(with_bias, GEMM_DTYPE)
    return _cache[key]


def kernel(x, weight, bias):
    from concourse.bass_utils import run_bass_kernel_spmd

    in_maps, with_bias = prepare_inputs(x, weight, bias)
    nc = get_nc(with_bias)
    res = run_bass_kernel_spmd(nc, in_maps, core_ids=list(range(NCORES)))
    out = np.concatenate([res.results[c]["out"] for c in range(NCORES)], axis=0)
    return out
